# revision 1
# baseline (speedup 1.0000x reference)
"""Single-head attention (no causal mask) on 8 Trainium2 NeuronCores.

Problem: inputs [32, 2048, 64], Wq/Wk/Wv [64, 64] (nn.Linear style, out = x @ W.T).
  q = x @ Wq^T ; k = x @ Wk^T ; v = x @ Wv^T
  out = softmax(q @ k^T / 8) @ v          # no causal mask in the reference

Sharding: data-parallel over the batch dim — 4 batch images per core, weights
replicated. No collectives; each core computes its own output slice.

Per-core design (per batch image):
  - Host pre-transposes x to xT [64, 2048]; weights host-transposed (+1/8 scale
    folded into Wq).
  - qT/kT [64h, 2048s] = W' @ xT on the PE (fp32r compute, bf16 storage);
    v [2048s, 64h] chunks via lhsT = xT chunk, stored bf16 with a ones column.
  - scores^T chunks [128k, 1024q] as bf16 matmuls (K=64).
  - exp on ScalarE straight out of PSUM (the per-core throughput floor:
    S*S*B/8 = 16.8M exps at 128/cycle @ 1.2 GHz).
  - U^T [65, 2048q] accumulated over k-chunks with lhsT = [v | 1], so row 64
    carries the softmax denominator.
  - U^T is stored to DRAM as-is; the final divide by row 64 and the
    [h, s] -> [s, h] transpose happen on host during unsharding.
"""

from contextlib import ExitStack

import numpy as np

import concourse.bass as bass
import concourse.mybir as mybir
import concourse.tile as tile
from concourse import bacc
from concourse.bass import ds, ts
from concourse.bass_utils import run_bass_kernel_spmd

F32 = mybir.dt.float32
F32R = mybir.dt.float32r
BF16 = mybir.dt.bfloat16
EXP = mybir.ActivationFunctionType.Exp

B, S, E, H = 32, 2048, 64, 64
NCORES = 8
BC = B // NCORES  # batches per core
NCH = S // 128  # k-chunks per batch
QH = 1024  # exp granularity along q (PSUM scores tile width)


def build_nc():
    nc = bacc.Bacc("TRN2", target_bir_lowering=False, debug=False)

    xt_d = nc.dram_tensor("xt", [BC, E, S], F32R, kind="ExternalInput").ap()
    wq_d = nc.dram_tensor("wq", [E, H], F32R, kind="ExternalInput").ap()
    wk_d = nc.dram_tensor("wk", [E, H], F32R, kind="ExternalInput").ap()
    wv_d = nc.dram_tensor("wv", [E, H], F32R, kind="ExternalInput").ap()
    out_d = nc.dram_tensor("out", [BC, H + 1, S], F32, kind="ExternalOutput").ap()

    ctx = ExitStack()
    with tile.TileContext(nc) as tc:
        with ctx:
            const = ctx.enter_context(tc.tile_pool(name="const", bufs=1))
            xt_pool = ctx.enter_context(tc.tile_pool(name="xt", bufs=2))
            qk_pool = ctx.enter_context(tc.tile_pool(name="qk", bufs=2))
            va_pool = ctx.enter_context(tc.tile_pool(name="va", bufs=2))
            ex_pool = ctx.enter_context(tc.tile_pool(name="ex", bufs=6))
            ut_pool = ctx.enter_context(tc.tile_pool(name="ut", bufs=2))
            ps_s = ctx.enter_context(tc.tile_pool(name="ps_s", bufs=2, space="PSUM"))
            ps_u = ctx.enter_context(tc.tile_pool(name="ps_u", bufs=1, space="PSUM"))

            ones = const.tile([128, NCH], F32, tag="ones")
            nc.gpsimd.memset(ones[:], 1.0)
            wq_s = const.tile([E, H], F32R, tag="wq")
            wk_s = const.tile([E, H], F32R, tag="wk")
            wv_s = const.tile([E, H], F32R, tag="wv")
            nc.sync.dma_start(wq_s[:], wq_d)
            nc.sync.dma_start(wk_s[:], wk_d)
            nc.sync.dma_start(wv_s[:], wv_d)

            def proj(b):
                """Load xT(b); compute qT, kT [64, S] bf16 and v_aug bf16."""
                xt_t = xt_pool.tile([E, S], F32R, tag="xt")
                nc.sync.dma_start(xt_t[:], xt_d[b])

                qT = qk_pool.tile([E, S], BF16, tag="qT")
                kT = qk_pool.tile([E, S], BF16, tag="kT")
                for w_s, dst in ((wq_s, qT), (wk_s, kT)):
                    for h2 in range(S // QH):
                        pp = ps_s.tile([128, QH], F32, tag="ps")
                        for j in range(QH // 512):
                            nc.tensor.matmul(
                                pp[0:E, ts(j, 512)],
                                w_s[:],
                                xt_t[:, ds(h2 * QH + j * 512, 512)],
                                start=True,
                                stop=True,
                            )
                        nc.vector.tensor_copy(
                            dst[:, ds(h2 * QH, QH)], pp[0:E, :]
                        )

                va = va_pool.tile([128, NCH * 65], BF16, tag="va")
                va_v = va[:].rearrange("p (c w) -> p c w", w=65)
                nc.vector.tensor_copy(
                    va_v[:, :, 64:65],
                    ones[:].rearrange("p (c w) -> p c w", w=1),
                )
                vp = ps_s.tile([128, QH], F32, tag="ps")
                for c in range(NCH):
                    nc.tensor.matmul(
                        vp[:, ts(c, 64)],
                        xt_t[:, ts(c, 128)],
                        wv_s[:],
                        start=True,
                        stop=True,
                    )
                nc.vector.tensor_copy(
                    va_v[:, :, 0:64],
                    vp[:].rearrange("p (c w) -> p c w", w=64),
                )
                return qT, kT, va

            def tail(b, ut_ps):
                """Evacuate U^T straight to DRAM (divide + transpose on host)."""
                ut_sb = ut_pool.tile([H + 1, S], F32, tag="ut")
                nc.vector.tensor_copy(ut_sb[:], ut_ps[0 : H + 1, :])
                nc.sync.dma_start(out_d[b], ut_sb[:])

            prev = None  # (b, ut_ps) pending tail
            for b in range(BC):
                qT, kT, va = proj(b)
                if prev is not None:
                    tail(*prev)
                ut_ps = ps_u.tile([H + 1, S], F32, tag="utp")
                va_v = va[:].rearrange("p (c w) -> p c w", w=65)
                for c in range(NCH):
                    for h2 in range(S // QH):
                        sc = ps_s.tile([128, QH], F32, tag="ps")
                        for j in range(QH // 512):
                            nc.tensor.matmul(
                                sc[:, ts(j, 512)],
                                kT[:, ts(c, 128)],
                                qT[:, ds(h2 * QH + j * 512, 512)],
                                start=True,
                                stop=True,
                            )
                        ex = ex_pool.tile([128, QH], BF16, tag="ex")
                        nc.scalar.activation(ex[:], sc[:], EXP)
                        for j in range(QH // 512):
                            nc.tensor.matmul(
                                ut_ps[0 : H + 1, ds(h2 * QH + j * 512, 512)],
                                va_v[:, c, :],
                                ex[:, ts(j, 512)],
                                start=(c == 0),
                                stop=(c == NCH - 1),
                            )
                prev = (b, ut_ps)
            tail(*prev)

    nc.compile()
    return nc


_NC = None


def _get_nc():
    global _NC
    if _NC is None:
        _NC = build_nc()
    return _NC


def _in_maps(inputs, Wq, Wk, Wv):
    xt = np.ascontiguousarray(np.transpose(inputs, (0, 2, 1)), dtype=np.float32)
    wq = np.ascontiguousarray(Wq.T, dtype=np.float32) / np.float32(np.sqrt(H))
    wk = np.ascontiguousarray(Wk.T, dtype=np.float32)
    wv = np.ascontiguousarray(Wv.T, dtype=np.float32)
    return [
        {"xt": xt[c * BC : (c + 1) * BC], "wq": wq, "wk": wk, "wv": wv}
        for c in range(NCORES)
    ]


def run(inputs, Wq, Wk, Wv, **spmd_kwargs):
    nc = _get_nc()
    res = run_bass_kernel_spmd(
        nc, _in_maps(inputs, Wq, Wk, Wv), core_ids=list(range(NCORES)), **spmd_kwargs
    )
    # Each core returns U^T [BC, 65, S]; row 64 is the softmax denominator.
    outs = []
    for r in res.results:
        ut = r["out"]
        outs.append(
            np.transpose(ut[:, :H, :] / ut[:, H : H + 1, :], (0, 2, 1))
        )
    return np.ascontiguousarray(np.concatenate(outs, 0), dtype=np.float32), res


def kernel(inputs, Wq, Wk, Wv):
    out, _ = run(inputs, Wq, Wk, Wv)
    return out



# revision 10
# speedup vs baseline: 1.4037x; 1.4037x over previous
"""Single-head attention (no causal mask) on 8 Trainium2 NeuronCores.

Problem: inputs [32, 2048, 64], Wq/Wk/Wv [64, 64] (nn.Linear style, out = x @ W.T).
  q = x @ Wq^T ; k = x @ Wk^T ; v = x @ Wv^T
  out = softmax(q @ k^T / 8) @ v          # no causal mask in the reference

Sharding: data-parallel over batch — 4 batch images per core, weights replicated.

Per-core design (v3):
  - Host pre-transposes x to xT [64, 2048]; weights host-transposed, 1/8 folded
    into Wq, and Wq/Wk DUPLICATED column-wise ([W|W], 64x128) so the projection
    materializes qT/kT twice: partitions 0-63 and 64-127 hold identical copies.
    Even k-chunks' score matmuls read the low half (PE row-tile T0), odd chunks
    the high half (T8); the two 64-row tiles stream concurrently (K=64 only
    fills half the array).
  - Warmup burst of dependency-free matmuls at t=0 flips the PE HAM clock gate
    to 8/8 (2.4 GHz) before the real pipeline starts; the steady-state loop is
    kept dense so it never re-throttles.
  - Flat chunk-step software pipeline: step s emits scores+exp for chunk s and
    the AV accumulation for chunk s-LAG, crossing batch boundaries without
    phase barriers. Projections for batch b+1 and the U^T evacuation of batch
    b-1 are spread into fixed chunk slots.
  - exp split between ScalarE (table exp) and VectorE (bf16 Schraudolph:
    bitcast(int16(x*184.665 + 16250.4)); ~+-3% on affected chunks, measured
    end-to-end error ~8e-3 absmax vs 2e-2 budget).
  - U^T [65, 2048] accumulated over k-chunks with lhsT = [v | 1] (row 64 =
    softmax denominator); divide + [h,s]->[s,h] transpose on host.
"""

import math
from contextlib import ExitStack

import numpy as np

import concourse.bass as bass
import concourse.mybir as mybir
import concourse.tile as tile
from concourse import bacc
from concourse.bass import ds, ts
from concourse.bass_utils import run_bass_kernel_spmd

F32 = mybir.dt.float32
F32R = mybir.dt.float32r
BF16 = mybir.dt.bfloat16
I16 = mybir.dt.int16
EXP = mybir.ActivationFunctionType.Exp
MULT = mybir.AluOpType.mult
ADD = mybir.AluOpType.add

B, S, E, H = 32, 2048, 64, 64
NCORES = 8
BC = B // NCORES  # batches per core
NCH = S // 128  # k-chunks per batch
QH = 1024  # exp instruction width along q
NQH = S // QH

# Schraudolph bf16 exp: bitcast(int16(x*SCHR_A + SCHR_B)) ~= exp(x)
SCHR_C = 5.6
SCHR_A = 128.0 / math.log(2.0)
SCHR_B = 127.0 * 128.0 - SCHR_C

# (c, h) exp instructions computed on VectorE via Schraudolph (rest: ScalarE).
EXP_DVE = {(c, 1) for c in (1, 2, 4, 6, 8, 10, 12, 13, 14)} | {(3, 0), (9, 0)}
LAG = 2  # AV trails scores by this many chunk-steps
WARMUP_MMS = 9


def build_nc():
    nc = bacc.Bacc("TRN2", target_bir_lowering=False, debug=False)

    xt_d = nc.dram_tensor("xt", [BC, E, S], F32R, kind="ExternalInput").ap()
    wq_d = nc.dram_tensor("wq", [E, 2 * H], F32R, kind="ExternalInput").ap()
    wk_d = nc.dram_tensor("wk", [E, 2 * H], F32R, kind="ExternalInput").ap()
    wv_d = nc.dram_tensor("wv", [E, H], F32R, kind="ExternalInput").ap()
    out_d = nc.dram_tensor("out", [BC, H + 1, S], F32, kind="ExternalOutput").ap()

    ctx = ExitStack()
    with tile.TileContext(nc) as tc:
        with ctx:
            const = ctx.enter_context(tc.tile_pool(name="const", bufs=1))
            xt_pool = ctx.enter_context(tc.tile_pool(name="xt", bufs=2))
            qk_pool = ctx.enter_context(tc.tile_pool(name="qk", bufs=4))
            va_pool = ctx.enter_context(tc.tile_pool(name="va", bufs=2))
            ex_pool = ctx.enter_context(tc.tile_pool(name="ex", bufs=14))
            ut_sb_pool = ctx.enter_context(tc.tile_pool(name="utsb", bufs=4))
            ps_a = ctx.enter_context(tc.tile_pool(name="ps_a", bufs=2, space="PSUM"))
            ps_u = ctx.enter_context(tc.tile_pool(name="ps_u", bufs=1, space="PSUM"))

            ones = const.tile([128, NCH], F32, tag="ones")
            nc.gpsimd.memset(ones[:], 1.0)
            scr_w = const.tile([E, 128], BF16, tag="scr_w")
            scr_x = const.tile([E, 512], BF16, tag="scr_x")
            nc.gpsimd.memset(scr_w[:], 0.0)
            nc.gpsimd.memset(scr_x[:], 0.0)
            wq_s = const.tile([E, 2 * H], F32R, tag="wq")
            wk_s = const.tile([E, 2 * H], F32R, tag="wk")
            wv_s = const.tile([E, H], F32R, tag="wv")
            nc.sync.dma_start(wq_s[:], wq_d)
            nc.sync.dma_start(wk_s[:], wk_d)
            nc.sync.dma_start(wv_s[:], wv_d)

            # HAM warmup: dependency-free back-to-back matmuls (~3.5us cold)
            warm = ps_a.tile([128, QH], F32, tag="ps")
            for _ in range(WARMUP_MMS):
                nc.tensor.matmul(
                    warm[:, 0:512], scr_w[:], scr_x[:], start=True, stop=True
                )

            def load_xt(b):
                xt_t = xt_pool.tile([E, S], F32R, tag="xt")
                nc.sync.dma_start(xt_t[:], xt_d[b])
                return xt_t

            def proj_qk(xt_t, w_s):
                """[128, S] bf16: rows 0-63 and 64-127 both hold W.T @ xT."""
                dst = qk_pool.tile([128, S], BF16, tag="qk")
                for h2 in range(S // QH):
                    pp = ps_a.tile([128, QH], F32, tag="ps")
                    for j in range(QH // 512):
                        nc.tensor.matmul(
                            pp[:, ts(j, 512)],
                            w_s[:],
                            xt_t[:, ds(h2 * QH + j * 512, 512)],
                            start=True,
                            stop=True,
                        )
                    if h2 == 0:
                        nc.scalar.copy(dst[:, ds(h2 * QH, QH)], pp[:])
                    else:
                        nc.vector.tensor_copy(dst[:, ds(h2 * QH, QH)], pp[:])
                return dst

            def proj_v(xt_t):
                """va [128, NCH*65] bf16: per chunk, 64 v-cols + ones col."""
                va = va_pool.tile([128, NCH * 65], BF16, tag="va")
                va_v = va[:].rearrange("p (c w) -> p c w", w=65)
                nc.vector.tensor_copy(
                    va_v[:, :, 64:65],
                    ones[:].rearrange("p (c w) -> p c w", w=1),
                )
                vp = ps_a.tile([128, QH], F32, tag="ps")
                for c in range(NCH):
                    nc.tensor.matmul(
                        vp[:, ts(c, 64)],
                        xt_t[:, ts(c, 128)],
                        wv_s[:],
                        start=True,
                        stop=True,
                    )
                nc.vector.tensor_copy(
                    va_v[:, :, 0:64],
                    vp[:].rearrange("p (c w) -> p c w", w=64),
                )
                return va

            def scores_chunk(qT, kT, c):
                """exp(scores^T) for k-chunk c -> [ex_h0, ex_h1] bf16 [128, QH]."""
                half = (c % 2) * 64
                exs = []
                for h in range(NQH):
                    sct = ps_a.tile([128, QH], F32, tag="ps")
                    for j in range(QH // 512):
                        nc.tensor.matmul(
                            sct[:, ts(j, 512)],
                            kT[:][ds(half, 64), ds(c * 128, 128)],
                            qT[:][ds(half, 64), ds(h * QH + j * 512, 512)],
                            start=True,
                            stop=True,
                        )
                    ex = ex_pool.tile([128, QH], BF16, tag="ex")
                    if (c, h) in EXP_DVE:
                        nc.vector.tensor_scalar(
                            ex[:].bitcast(I16), sct[:], SCHR_A, SCHR_B, MULT, ADD
                        )
                    else:
                        nc.scalar.activation(ex[:], sct[:], EXP)
                    exs.append(ex)
                return exs

            def av_chunk(ut_ps, va, exs_c, c):
                va_v = va[:].rearrange("p (c w) -> p c w", w=65)
                for h in range(NQH):
                    for j in range(QH // 512):
                        nc.tensor.matmul(
                            ut_ps[0 : H + 1, ds(h * QH + j * 512, 512)],
                            va_v[:, c, :],
                            exs_c[h][:, ts(j, 512)],
                            start=(c == 0),
                            stop=(c == NCH - 1),
                        )

            def tail(b, ut_ps):
                """Evacuate U^T halves (ScalarE + VectorE) and DMA out."""
                ut_sb = ut_sb_pool.tile([H + 1, S], F32, tag="ut")
                nc.scalar.copy(ut_sb[:, 0:QH], ut_ps[0 : H + 1, 0:QH])
                nc.vector.tensor_copy(ut_sb[:, QH:S], ut_ps[0 : H + 1, QH:S])
                nc.sync.dma_start(out_d[b], ut_sb[:])

            # prologue: batch 0 projections
            xt_t = load_xt(0)
            qTs = {0: proj_qk(xt_t, wq_s)}
            kTs = {0: proj_qk(xt_t, wk_s)}
            vas = {0: proj_v(xt_t)}
            xts = {0: xt_t}

            exs_all = {}  # (b, c) -> [ex_h0, ex_h1]
            ut_tiles = {}
            pending_tail = {}
            for s in range(BC * NCH + LAG):
                if s < BC * NCH:
                    b, c = divmod(s, NCH)
                    exs_all[(b, c)] = scores_chunk(qTs[b], kTs[b], c)
                    if b + 1 < BC:
                        if c == 1:
                            xts[b + 1] = load_xt(b + 1)
                        elif c == 3:
                            qTs[b + 1] = proj_qk(xts[b + 1], wq_s)
                        elif c == 7:
                            kTs[b + 1] = proj_qk(xts[b + 1], wk_s)
                        elif c == 11:
                            vas[b + 1] = proj_v(xts[b + 1])
                    # drop handles no longer needed
                    if c == NCH - 1:
                        xts.pop(b, None)
                av = s - LAG
                if av >= 0:
                    ba, ca = divmod(av, NCH)
                    if ca == 0:
                        ut_ps = ps_u.tile([H + 1, S], F32, tag="utp")
                        ut_tiles[ba] = ut_ps
                    av_chunk(ut_tiles[ba], vas[ba], exs_all.pop((ba, ca)), ca)
                    if ca == NCH - 1:
                        tail(ba, ut_tiles.pop(ba))

    nc.compile()
    return nc


_NC = None


def _get_nc():
    global _NC
    if _NC is None:
        _NC = build_nc()
    return _NC


def _in_maps(inputs, Wq, Wk, Wv):
    xt = np.ascontiguousarray(np.transpose(inputs, (0, 2, 1)), dtype=np.float32)
    wq1 = Wq.T.astype(np.float32) / np.float32(np.sqrt(H))
    wq = np.ascontiguousarray(np.concatenate([wq1, wq1], axis=1))
    wk1 = Wk.T.astype(np.float32)
    wk = np.ascontiguousarray(np.concatenate([wk1, wk1], axis=1))
    wv = np.ascontiguousarray(Wv.T, dtype=np.float32)
    return [
        {"xt": xt[c * BC : (c + 1) * BC], "wq": wq, "wk": wk, "wv": wv}
        for c in range(NCORES)
    ]


def run(inputs, Wq, Wk, Wv, **spmd_kwargs):
    nc = _get_nc()
    res = run_bass_kernel_spmd(
        nc, _in_maps(inputs, Wq, Wk, Wv), core_ids=list(range(NCORES)), **spmd_kwargs
    )
    # Each core returns U^T [BC, 65, S]; row 64 is the softmax denominator.
    outs = []
    for r in res.results:
        ut = r["out"]
        outs.append(
            np.transpose(ut[:, :H, :] / ut[:, H : H + 1, :], (0, 2, 1))
        )
    return np.ascontiguousarray(np.concatenate(outs, 0), dtype=np.float32), res


def kernel(inputs, Wq, Wk, Wv):
    out, _ = run(inputs, Wq, Wk, Wv)
    return out


# revision 15
# speedup vs baseline: 1.5144x; 1.0789x over previous
"""Single-head attention (no causal mask) on 8 Trainium2 NeuronCores.

Problem: inputs [32, 2048, 64], Wq/Wk/Wv [64, 64] (nn.Linear style, out = x @ W.T).
  q = x @ Wq^T ; k = x @ Wk^T ; v = x @ Wv^T
  out = softmax(q @ k^T / 8) @ v          # no causal mask in the reference

Sharding: data-parallel over batch — 4 batch images per core, weights replicated.

Per-core design (v3):
  - Host pre-transposes x to xT [64, 2048]; weights host-transposed, 1/8 folded
    into Wq, and Wq/Wk DUPLICATED column-wise ([W|W], 64x128) so the projection
    materializes qT/kT twice: partitions 0-63 and 64-127 hold identical copies.
    Even k-chunks' score matmuls read the low half (PE row-tile T0), odd chunks
    the high half (T8); the two 64-row tiles stream concurrently (K=64 only
    fills half the array).
  - Warmup burst of dependency-free matmuls at t=0 flips the PE HAM clock gate
    to 8/8 (2.4 GHz) before the real pipeline starts; the steady-state loop is
    kept dense so it never re-throttles.
  - Flat chunk-step software pipeline: step s emits scores+exp for chunk s and
    the AV accumulation for chunk s-LAG, crossing batch boundaries without
    phase barriers. Projections for batch b+1 and the U^T evacuation of batch
    b-1 are spread into fixed chunk slots.
  - exp split between ScalarE (table exp) and VectorE (bf16 Schraudolph:
    bitcast(int16(x*184.665 + 16250.4)); ~+-3% on affected chunks, measured
    end-to-end error ~8e-3 absmax vs 2e-2 budget).
  - U^T [65, 2048] accumulated over k-chunks with lhsT = [v | 1] (row 64 =
    softmax denominator); divide + [h,s]->[s,h] transpose on host.
"""

import math
from contextlib import ExitStack

import numpy as np

import concourse.bass as bass
import concourse.mybir as mybir
import concourse.tile as tile
from concourse import bacc
from concourse.bass import ds, ts
from concourse.bass_utils import run_bass_kernel_spmd

F32 = mybir.dt.float32
F32R = mybir.dt.float32r
BF16 = mybir.dt.bfloat16
I16 = mybir.dt.int16
EXP = mybir.ActivationFunctionType.Exp
MULT = mybir.AluOpType.mult
ADD = mybir.AluOpType.add

B, S, E, H = 32, 2048, 64, 64
NCORES = 8
BC = B // NCORES  # batches per core
NCH = S // 128  # k-chunks per batch
QH = 1024  # exp instruction width along q
NQH = S // QH

# Schraudolph bf16 exp: bitcast(int16(x*SCHR_A + SCHR_B)) ~= exp(x)
SCHR_C = 5.6
SCHR_A = 128.0 / math.log(2.0)
SCHR_B = 127.0 * 128.0 - SCHR_C

# (c, h) exp instructions computed on VectorE via Schraudolph (rest: ScalarE).
EXP_DVE = {(c, 1) for c in range(NCH) if c not in (0, NCH - 1)}
LAG = 3  # AV trails scores by this many chunk-steps
WARMUP_MMS = 20


def build_nc():
    nc = bacc.Bacc("TRN2", target_bir_lowering=False, debug=False)

    xt_d = nc.dram_tensor("xt", [BC, E, S], F32R, kind="ExternalInput").ap()
    wq_d = nc.dram_tensor("wq", [E, 2 * H], F32R, kind="ExternalInput").ap()
    wk_d = nc.dram_tensor("wk", [E, 2 * H], F32R, kind="ExternalInput").ap()
    wv_d = nc.dram_tensor("wv", [E, H], F32R, kind="ExternalInput").ap()
    out_d = nc.dram_tensor("out", [BC, H + 1, S], F32, kind="ExternalOutput").ap()

    ctx = ExitStack()
    with tile.TileContext(nc) as tc:
        with ctx:
            const = ctx.enter_context(tc.tile_pool(name="const", bufs=1))
            xt_pool = ctx.enter_context(tc.tile_pool(name="xt", bufs=3))
            qk_pool = ctx.enter_context(tc.tile_pool(name="qk", bufs=4))
            va_pool = ctx.enter_context(tc.tile_pool(name="va", bufs=2))
            ex_pool = ctx.enter_context(tc.tile_pool(name="ex", bufs=14))
            ut_sb_pool = ctx.enter_context(tc.tile_pool(name="utsb", bufs=4))
            ps_a = ctx.enter_context(tc.tile_pool(name="ps_a", bufs=2, space="PSUM"))
            ps_u = ctx.enter_context(tc.tile_pool(name="ps_u", bufs=1, space="PSUM"))

            ones = const.tile([128, NCH], F32, tag="ones")
            nc.gpsimd.memset(ones[:], 1.0)
            # scratch operands for the warmup burst (results discarded)
            scr_w = const.tile([E, 128], BF16, tag="scr_w")
            scr_x = const.tile([E, 512], BF16, tag="scr_x")
            nc.gpsimd.memset(scr_w[:], 0.0)
            nc.gpsimd.memset(scr_x[:], 0.0)
            wq_s = const.tile([E, 2 * H], F32R, tag="wq")
            wk_s = const.tile([E, 2 * H], F32R, tag="wk")
            wv_s = const.tile([E, H], F32R, tag="wv")
            nc.sync.dma_start(wq_s[:], wq_d)
            nc.sync.dma_start(wk_s[:], wk_d)
            nc.sync.dma_start(wv_s[:], wv_d)

            # HAM warmup: dependency-free back-to-back matmuls. The clock-gate
            # flips to 8/8 only after one FULL 4096-cycle window of activity;
            # the window is free-running, so burn ~2 windows' worth.
            warm = ps_a.tile([128, QH], F32, tag="ps")
            for _ in range(WARMUP_MMS):
                nc.tensor.matmul(
                    warm[:, 0:512], scr_w[:], scr_x[:], start=True, stop=True
                )

            def load_xt(b):
                xt_t = xt_pool.tile([E, S], F32R, tag="xt")
                nc.sync.dma_start(xt_t[:], xt_d[b])
                return xt_t

            def proj_qk(xt_t, w_s):
                """[128, S] bf16: rows 0-63 and 64-127 both hold W.T @ xT."""
                dst = qk_pool.tile([128, S], BF16, tag="qk")
                for h2 in range(S // QH):
                    pp = ps_a.tile([128, QH], F32, tag="ps")
                    for j in range(QH // 512):
                        nc.tensor.matmul(
                            pp[:, ts(j, 512)],
                            w_s[:],
                            xt_t[:, ds(h2 * QH + j * 512, 512)],
                            start=True,
                            stop=True,
                        )
                    if h2 == 0:
                        nc.scalar.copy(dst[:, ds(h2 * QH, QH)], pp[:])
                    else:
                        nc.vector.tensor_copy(dst[:, ds(h2 * QH, QH)], pp[:])
                return dst

            def proj_v(xt_t):
                """va [128, NCH*65] bf16: per chunk, 64 v-cols + ones col."""
                va = va_pool.tile([128, NCH * 65], BF16, tag="va")
                va_v = va[:].rearrange("p (c w) -> p c w", w=65)
                nc.vector.tensor_copy(
                    va_v[:, :, 64:65],
                    ones[:].rearrange("p (c w) -> p c w", w=1),
                )
                vp = ps_a.tile([128, QH], F32, tag="ps")
                for c in range(NCH):
                    nc.tensor.matmul(
                        vp[:, ts(c, 64)],
                        xt_t[:, ts(c, 128)],
                        wv_s[:],
                        start=True,
                        stop=True,
                    )
                nc.vector.tensor_copy(
                    va_v[:, :, 0:64],
                    vp[:].rearrange("p (c w) -> p c w", w=64),
                )
                return va

            def scores_chunk(qT, kT, c):
                """exp(scores^T) for k-chunk c -> [ex_h0, ex_h1] bf16 [128, QH]."""
                half = (c % 2) * 64
                exs = []
                for h in range(NQH):
                    sct = ps_a.tile([128, QH], F32, tag="ps")
                    for j in range(QH // 512):
                        nc.tensor.matmul(
                            sct[:, ts(j, 512)],
                            kT[:][ds(half, 64), ds(c * 128, 128)],
                            qT[:][ds(half, 64), ds(h * QH + j * 512, 512)],
                            start=True,
                            stop=True,
                        )
                    ex = ex_pool.tile([128, QH], BF16, tag="ex")
                    if (c, h) in EXP_DVE:
                        nc.vector.tensor_scalar(
                            ex[:].bitcast(I16), sct[:], SCHR_A, SCHR_B, MULT, ADD
                        )
                    else:
                        nc.scalar.activation(ex[:], sct[:], EXP)
                    exs.append(ex)
                return exs

            def av_chunk(ut_ps, va, exs_c, c):
                va_v = va[:].rearrange("p (c w) -> p c w", w=65)
                for h in range(NQH):
                    for j in range(QH // 512):
                        nc.tensor.matmul(
                            ut_ps[0 : H + 1, ds(h * QH + j * 512, 512)],
                            va_v[:, c, :],
                            exs_c[h][:, ts(j, 512)],
                            start=(c == 0),
                            stop=(c == NCH - 1),
                        )

            def tail(b, ut_ps):
                """Evacuate U^T halves (ScalarE + VectorE) and DMA out."""
                ut_sb = ut_sb_pool.tile([H + 1, S], F32, tag="ut")
                nc.scalar.copy(ut_sb[:, 0:QH], ut_ps[0 : H + 1, 0:QH])
                nc.vector.tensor_copy(ut_sb[:, QH:S], ut_ps[0 : H + 1, QH:S])
                nc.sync.dma_start(out_d[b], ut_sb[:])

            # prologue: batch 0 projections, xt prefetch for 0 and 1
            xts = {0: load_xt(0), 1: load_xt(1)}
            qTs = {0: proj_qk(xts[0], wq_s)}
            kTs = {0: proj_qk(xts[0], wk_s)}
            vas = {0: proj_v(xts[0])}

            exs_all = {}  # (b, c) -> [ex_h0, ex_h1]
            ut_tiles = {}
            due_tail = None
            for s in range(BC * NCH + LAG):
                # evacuation copies first so ACT/DVE run them ahead of exps
                if due_tail is not None:
                    tail(*due_tail)
                    due_tail = None
                if s < BC * NCH:
                    b, c = divmod(s, NCH)
                    exs_all[(b, c)] = scores_chunk(qTs[b], kTs[b], c)
                    if b + 1 < BC:
                        if c == 3:
                            qTs[b + 1] = proj_qk(xts[b + 1], wq_s)
                        elif c == 7:
                            kTs[b + 1] = proj_qk(xts[b + 1], wk_s)
                        elif c == 11:
                            vas[b + 1] = proj_v(xts[b + 1])
                    if c == 8 and b + 2 < BC:
                        xts[b + 2] = load_xt(b + 2)
                    if c == NCH - 1:
                        xts.pop(b, None)
                av = s - LAG
                if av >= 0:
                    ba, ca = divmod(av, NCH)
                    if ca == 0:
                        ut_ps = ps_u.tile([H + 1, S], F32, tag="utp")
                        ut_tiles[ba] = ut_ps
                    av_chunk(ut_tiles[ba], vas[ba], exs_all.pop((ba, ca)), ca)
                    if ca == NCH - 1:
                        due_tail = (ba, ut_tiles.pop(ba))
            if due_tail is not None:
                tail(*due_tail)

    nc.compile()
    return nc


_NC = None


def _get_nc():
    global _NC
    if _NC is None:
        _NC = build_nc()
    return _NC


def _in_maps(inputs, Wq, Wk, Wv):
    xt = np.ascontiguousarray(np.transpose(inputs, (0, 2, 1)), dtype=np.float32)
    wq1 = Wq.T.astype(np.float32) / np.float32(np.sqrt(H))
    wq = np.ascontiguousarray(np.concatenate([wq1, wq1], axis=1))
    wk1 = Wk.T.astype(np.float32)
    wk = np.ascontiguousarray(np.concatenate([wk1, wk1], axis=1))
    wv = np.ascontiguousarray(Wv.T, dtype=np.float32)
    return [
        {"xt": xt[c * BC : (c + 1) * BC], "wq": wq, "wk": wk, "wv": wv}
        for c in range(NCORES)
    ]


def run(inputs, Wq, Wk, Wv, **spmd_kwargs):
    nc = _get_nc()
    res = run_bass_kernel_spmd(
        nc, _in_maps(inputs, Wq, Wk, Wv), core_ids=list(range(NCORES)), **spmd_kwargs
    )
    # Each core returns U^T [BC, 65, S]; row 64 is the softmax denominator.
    outs = []
    for r in res.results:
        ut = r["out"]
        outs.append(
            np.transpose(ut[:, :H, :] / ut[:, H : H + 1, :], (0, 2, 1))
        )
    return np.ascontiguousarray(np.concatenate(outs, 0), dtype=np.float32), res


def kernel(inputs, Wq, Wk, Wv):
    out, _ = run(inputs, Wq, Wk, Wv)
    return out
